# revision 2
# baseline (speedup 1.0000x reference)
"""MoE (DeepSeek-style) routed+shared expert forward on 8 TRN2 NeuronCores.

Strategy (expert-parallel, host-side dispatch):
  - Host computes the gate (softmax + top-2) in float64 and gathers each
    expert's routed tokens; core e processes expert e's tokens (padded to
    capacity C) plus a 1/8 token-slice through the shared-expert MLP.
  - Routed expert matmuls run in fp8(e4m3) with DoubleRow perf mode (two
    128-row k-tiles per instruction, ~1.8x the bf16 rate). The routed
    path contributes only ~23% of the output norm (gate weights ~0.23),
    so fp8 quantization error stays within the accuracy budget.
  - The shared expert (97% of the output norm) runs in float16 (same PE
    rate as bf16, 8x finer mantissa).
  - Dequantization scales fold into activation scale APs; host packs a
    per-core scale/bias table so one SPMD program serves all cores.

ROUTED_MODE:
  "1t"    - all routed matmuls 1-term fp8 (fastest)
  "l1x2"  - layer-1 compensates x with an fp8 residual (2 chains)
  "l2f16" - layer-2 (w2) in f16 instead of fp8
"""

import sys

if "/opt/trn_rl_repo" not in sys.path:
    sys.path.insert(0, "/opt/trn_rl_repo")

import ml_dtypes
import numpy as np

import concourse.bass as bass
import concourse.tile as tile
from concourse import bacc, mybir
from concourse import bass_utils

B, S, DIM = 4, 2048, 1024
T = B * S
INTER = 1024
E = 8
TOPK = 2
ROUTE_SCALE = 1.0
SHARED_INTER = 2048
N_CORES = 8
TS = T // N_CORES          # shared-expert tokens per core
NBLK = 512                 # token block (one fp32 PSUM bank)

F32 = mybir.dt.float32
BF16 = mybir.dt.bfloat16
F16 = mybir.dt.float16
F8 = mybir.dt.float8e4
DR = mybir.MatmulPerfMode.DoubleRow
SILU = mybir.ActivationFunctionType.Silu
IDENT = mybir.ActivationFunctionType.Identity

F8NP = ml_dtypes.float8_e4m3   # IEEE e4m3, max 240

S_X = 16.0    # x quant scale (x absmax ~5.3 -> 85)
S_H = 16.0    # routed intermediate-h quant scale (h absmax ~6 -> 96)

ROUTED_MODE = "1t"

ND = DIM // 128
NI = INTER // 128
NS = SHARED_INTER // 128

_program_cache = {}


def _blocks(total, blk=NBLK):
    out, o = [], 0
    while o < total:
        n = min(blk, total - o)
        out.append((o, n))
        o += n
    return out


def build_program(C, mode):
    nc = bacc.Bacc("TRN2", target_bir_lowering=False, debug=False,
                   num_devices=N_CORES)

    def din(name, shape, dt):
        return nc.dram_tensor(name, shape, dt, kind="ExternalInput").ap()

    xe_h = din("xe_h", (DIM, C), F8)
    xe_l = din("xe_l", (DIM, C), F8) if mode == "l1x2" else None
    w1h = din("w1h", (DIM, INTER), F8)
    w3h = din("w3h", (DIM, INTER), F8)
    w2h = din("w2h", (INTER, DIM), F16 if mode == "l2f16" else F8)
    xs = din("xs", (DIM, TS), F16)
    ws1 = din("ws1", (DIM, SHARED_INTER), F16)
    ws3 = din("ws3", (DIM, SHARED_INTER), F16)
    ws2 = din("ws2", (SHARED_INTER, DIM), F16)
    scb = din("scb", (128, 80), F32)
    ye = nc.dram_tensor("ye", (DIM, C), BF16, kind="ExternalOutput").ap()
    ys = nc.dram_tensor("ys", (DIM, TS), F16, kind="ExternalOutput").ap()
    gate_scr = nc.dram_tensor("gate_scr", (128, 8), F8, kind="Internal").ap()

    def r_in(ap):   # (dk p) n -> p dk n
        return None if ap is None else \
            ap.rearrange("(dk p) n -> p dk n", p=128)

    xe_hr, xe_lr, xs_r = r_in(xe_h), r_in(xe_l), r_in(xs)
    w1r, w3r, w2r = r_in(w1h), r_in(w3h), r_in(w2h)
    ws1r, ws3r, ws2r = r_in(ws1), r_in(ws3), r_in(ws2)
    ye_r = ye.rearrange("(md p) c -> p md c", p=128)
    ys_r = ys.rearrange("(md p) c -> p md c", p=128)

    with tile.TileContext(nc) as tc:
        from contextlib import ExitStack
        es = ExitStack()
        pers = es.enter_context(tc.tile_pool(name="pers", bufs=1,
                                             side="right"))
        scb_sb = pers.tile([128, 80], F32, tag="scb")
        nc.sync.dma_start(scb_sb[:], scb[:])
        inv1, inv3, invy = (scb_sb[:, 0:1], scb_sb[:, 1:2], scb_sb[:, 2:3])
        b1_t = scb_sb[:, 8:8 + NI]
        b3_t = scb_sb[:, 16:16 + NI]       # pre-multiplied by S_H
        b2_t = scb_sb[:, 24:24 + ND]
        bs1_t = scb_sb[:, 32:32 + NS]
        bs3_t = scb_sb[:, 48:48 + NS]
        bs2_t = scb_sb[:, 64:64 + ND]

        def dma_ktiles(dst, src, nk, eng=None):
            eng = eng or nc.sync
            for dk in range(nk):
                eng.dma_start(dst[:, dk, :], src[:, dk, :])

        # shared-expert stationary data lives in a persistent pool and is
        # prefetched while the routed phase computes
        xs_sb = pers.tile([128, ND, TS], F16, tag="xs")
        ws1_sb = pers.tile([128, ND, SHARED_INTER], F16, tag="ws1")
        ws3_sb = pers.tile([128, ND, SHARED_INTER], F16, tag="ws3")
        ws2_sb = pers.tile([128, NS, DIM], F16, tag="ws2")

        # ================= Phase 1: routed expert (fp8 DoubleRow) ======
        with tc.tile_pool(name="wexp", bufs=1) as wpool, \
             tc.tile_pool(name="xep", bufs=1) as xpool, \
             tc.tile_pool(name="hbp", bufs=2) as hpool, \
             tc.tile_pool(name="tmp", bufs=2) as tpool, \
             tc.tile_pool(name="yout", bufs=3) as ypool, \
             tc.tile_pool(name="ps", bufs=2, space="PSUM") as pspool:
            blocks = _blocks(C)
            xeh_sb = xpool.tile([128, ND, C], F8, tag="xe_h")
            w1_sb = wpool.tile([128, ND, INTER], F8, tag="w1h")
            w3_sb = wpool.tile([128, ND, INTER], F8, tag="w3h")
            # weights on the sync ring, x tokens on the gpsimd ring: the
            # two rings fetch concurrently. Full-C x chunks keep DMA
            # segments >= 2KB (per-block chunks would be 512B strided
            # segments that crawl at ~60GB/s).
            for dk in range(ND):
                nc.sync.dma_start(w1_sb[:, dk, :], w1r[:, dk, :])
                nc.sync.dma_start(w3_sb[:, dk, :], w3r[:, dk, :])
                if dk < ND // 2:
                    nc.sync.dma_start(xeh_sb[:, dk, :], xe_hr[:, dk, :])
                else:
                    nc.gpsimd.dma_start(xeh_sb[:, dk, :], xe_hr[:, dk, :])
            if mode == "l1x2":
                xel_sb = xpool.tile([128, ND, C], F8, tag="xe_l")
                dma_ktiles(xel_sb, xe_lr, ND, eng=nc.gpsimd)
            w2dt = F16 if mode == "l2f16" else F8
            w2_sb = wpool.tile([128, NI, DIM], w2dt, tag="w2h")
            dma_ktiles(w2_sb, w2r, NI)
            # prefetch shared-phase data on the gpsimd ring, gated behind
            # the last routed inputs (both rings) so it doesn't steal HBM
            # bandwidth from the routed phase's startup
            nc.gpsimd.dma_start(gate_scr[:, 0:4], xeh_sb[:, ND - 1, C - 4:C])
            if mode != "l2f16":
                nc.gpsimd.dma_start(gate_scr[:, 4:8], w2_sb[:, NI - 1, 0:4])
            dma_ktiles(xs_sb, xs_r, ND, eng=nc.gpsimd)
            dma_ktiles(ws1_sb, ws1r, ND, eng=nc.gpsimd)
            dma_ktiles(ws3_sb, ws3r, ND, eng=nc.gpsimd)
            dma_ktiles(ws2_sb, ws2r, NS, eng=nc.gpsimd)

            hdt = F16 if mode == "l2f16" else F8
            for (off, n) in _blocks(C):
                h_sb = hpool.tile([128, NI, n], hdt, tag="h", name="h",
                                  padded_shape=[128, NI, NBLK])
                for mi in range(NI):
                    msl = slice(mi * 128, (mi + 1) * 128)
                    ps1 = pspool.tile([128, n], F32, tag="ps1",
                                      padded_shape=[128, NBLK])
                    ps3 = pspool.tile([128, n], F32, tag="ps3",
                                      padded_shape=[128, NBLK])
                    for ps, w_sb_ in ((ps1, w1_sb), (ps3, w3_sb)):
                        xs_list = [xeh_sb] if mode != "l1x2" \
                            else [xeh_sb, xel_sb]
                        nmm = len(xs_list) * (ND // 2)
                        i = 0
                        for xt in xs_list:
                            for dk in range(0, ND, 2):
                                nc.tensor.matmul(
                                    ps[:], w_sb_[:, dk:dk + 2, msl],
                                    xt[:, dk:dk + 2, off:off + n],
                                    start=(i == 0), stop=(i == nmm - 1),
                                    perf_mode=DR)
                                i += 1
                    t1 = tpool.tile([128, n], BF16, tag="t1", name="t1",
                                    padded_shape=[128, NBLK])
                    nc.scalar.activation(t1[:], ps1[:], SILU,
                                         bias=b1_t[:, mi:mi + 1], scale=inv1)
                    t3 = tpool.tile([128, n], BF16, tag="t3", name="t3",
                                    padded_shape=[128, NBLK])
                    nc.scalar.activation(t3[:], ps3[:], IDENT,
                                         bias=b3_t[:, mi:mi + 1], scale=inv3)
                    nc.vector.tensor_mul(h_sb[:, mi, :], t1[:], t3[:])
                for md in range(ND):
                    msl = slice(md * 128, (md + 1) * 128)
                    psy = pspool.tile([128, n], F32, tag="psy",
                                      padded_shape=[128, NBLK])
                    if mode == "l2f16":
                        for mi in range(NI):
                            nc.tensor.matmul(
                                psy[:], w2_sb[:, mi, msl], h_sb[:, mi, :],
                                start=(mi == 0), stop=(mi == NI - 1))
                    else:
                        for mi in range(0, NI, 2):
                            nc.tensor.matmul(
                                psy[:], w2_sb[:, mi:mi + 2, msl],
                                h_sb[:, mi:mi + 2, :],
                                start=(mi == 0), stop=(mi == NI - 2),
                                perf_mode=DR)
                    yt = ypool.tile([128, n], BF16, tag="yt", name="yt",
                                    padded_shape=[128, NBLK])
                    nc.scalar.activation(yt[:], psy[:], IDENT,
                                         bias=b2_t[:, md:md + 1], scale=invy)
                    nc.sync.dma_start(ye_r[:, md, off:off + n], yt[:])

            # ========== Phase 2: shared expert (f16), same pools ========
            for (off, n) in _blocks(TS):
                hs_sb = hpool.tile([128, NS, n], F16, tag="hs", name="hs",
                                   padded_shape=[128, NS, NBLK])
                for mi in range(NS):
                    msl = slice(mi * 128, (mi + 1) * 128)
                    ps1 = pspool.tile([128, n], F32, tag="ps1",
                                      padded_shape=[128, NBLK])
                    ps3 = pspool.tile([128, n], F32, tag="ps3",
                                      padded_shape=[128, NBLK])
                    for ps, w_sb_ in ((ps1, ws1_sb), (ps3, ws3_sb)):
                        for dk in range(ND):
                            nc.tensor.matmul(
                                ps[:], w_sb_[:, dk, msl],
                                xs_sb[:, dk, off:off + n],
                                start=(dk == 0), stop=(dk == ND - 1))
                    t1 = tpool.tile([128, n], F16, tag="t1s", name="t1s",
                                    padded_shape=[128, NBLK])
                    nc.scalar.activation(t1[:], ps1[:], SILU,
                                         bias=bs1_t[:, mi:mi + 1])
                    t3 = tpool.tile([128, n], F16, tag="t3s", name="t3s",
                                    padded_shape=[128, NBLK])
                    nc.scalar.activation(t3[:], ps3[:], IDENT,
                                         bias=bs3_t[:, mi:mi + 1])
                    nc.vector.tensor_mul(hs_sb[:, mi, :], t1[:], t3[:])
                for md in range(ND):
                    msl = slice(md * 128, (md + 1) * 128)
                    psy = pspool.tile([128, n], F32, tag="psy",
                                      padded_shape=[128, NBLK])
                    for mi in range(NS):
                        nc.tensor.matmul(
                            psy[:], ws2_sb[:, mi, msl], hs_sb[:, mi, :],
                            start=(mi == 0), stop=(mi == NS - 1))
                    yt = ypool.tile([128, n], F16, tag="yts", name="yts",
                                    padded_shape=[128, NBLK])
                    nc.scalar.activation(yt[:], psy[:], IDENT,
                                         bias=bs2_t[:, md:md + 1])
                    nc.sync.dma_start(ys_r[:, md, off:off + n], yt[:])
        es.close()

    nc.compile()
    return nc


def _q8(a):
    return np.clip(a, -240.0, 240.0).astype(F8NP)


def _pow2_scale(a, target=192.0):
    m = float(np.abs(a).max())
    return float(2.0 ** np.floor(np.log2(target / max(m, 1e-30))))


def _pack_cols(vec, tab, c0):
    k = len(vec) // 128
    tab[:, c0:c0 + k] = vec.reshape(k, 128).T


def _gate_host(xt, gate_w, gate_b):
    logits = xt.astype(np.float64) @ gate_w.astype(np.float64).T \
        + gate_b.astype(np.float64)
    m = logits.max(axis=-1, keepdims=True)
    p = np.exp(logits - m)
    scores = p / p.sum(axis=-1, keepdims=True)
    order = np.argsort(-scores, axis=1, kind="stable")
    top_i = order[:, :TOPK]
    top_w = (np.take_along_axis(scores, top_i, axis=1)
             * ROUTE_SCALE).astype(np.float32)
    return top_i, top_w


def run(inputs, trace=False):
    x = np.ascontiguousarray(np.asarray(inputs["x"], dtype=np.float32))
    f32 = lambda k: np.asarray(inputs[k], dtype=np.float32)
    gate_w, gate_b = f32("gate_w"), f32("gate_b")
    w1, b1, w3, b3 = f32("w1"), f32("b1"), f32("w3"), f32("b3")
    w2, b2 = f32("w2"), f32("b2")
    ws1, bs1, ws3, bs3 = f32("ws1"), f32("bs1"), f32("ws3"), f32("bs3")
    ws2, bs2 = f32("ws2"), f32("bs2")

    xt = x.reshape(T, DIM)
    top_i, top_w = _gate_host(xt, gate_w, gate_b)

    idx, wgt = [], []
    for e in range(E):
        toks = np.nonzero((top_i == e).any(axis=1))[0]
        idx.append(toks)
        slot = (top_i[toks] == e)
        wgt.append(top_w[toks][slot])

    cmax = max(len(i) for i in idx)
    C = max(256, (cmax + 31) & ~31)

    xT = np.ascontiguousarray(xt.T)
    x_hi = _q8(xT * S_X)
    if ROUTED_MODE == "l1x2":
        x_lo = _q8(xT * S_X - x_hi.astype(np.float32))
    xs_f16 = xT.astype(np.float16)

    ws1t = np.ascontiguousarray(ws1.T).astype(np.float16)
    ws3t = np.ascontiguousarray(ws3.T).astype(np.float16)
    ws2t = np.ascontiguousarray(ws2.T).astype(np.float16)

    in_maps = []
    for e in range(E):
        n_e = len(idx[e])
        xe_h = np.zeros((DIM, C), F8NP)
        xe_h[:, :n_e] = x_hi[:, idx[e]]
        sl = slice(TS * e, TS * (e + 1))
        s_w1, s_w3 = _pow2_scale(w1[e]), _pow2_scale(w3[e])
        w1h_ = _q8(np.ascontiguousarray(w1[e].T) * s_w1)
        w3h_ = _q8(np.ascontiguousarray(w3[e].T) * s_w3)
        if ROUTED_MODE == "l2f16":
            s_w2 = 1.0
            w2h_ = np.ascontiguousarray(w2[e].T).astype(np.float16)
        else:
            s_w2 = _pow2_scale(w2[e])
            w2h_ = _q8(np.ascontiguousarray(w2[e].T) * s_w2)

        tab = np.zeros((128, 80), np.float32)
        tab[:, 0] = 1.0 / (S_X * s_w1)
        tab[:, 1] = S_H / (S_X * s_w3)
        tab[:, 2] = 1.0 / (S_H * s_w2)
        if ROUTED_MODE == "l2f16":
            tab[:, 1] = 1.0 / (S_X * s_w3)
            tab[:, 2] = 1.0
        _pack_cols(b1[e], tab, 8)
        _pack_cols(b3[e] * (1.0 if ROUTED_MODE == "l2f16" else S_H), tab, 16)
        _pack_cols(b2[e], tab, 24)
        _pack_cols(bs1, tab, 32)
        _pack_cols(bs3, tab, 48)
        _pack_cols(bs2, tab, 64)

        im = {
            "xe_h": xe_h,
            "xs": np.ascontiguousarray(xs_f16[:, sl]),
            "w1h": w1h_, "w3h": w3h_, "w2h": w2h_,
            "ws1": ws1t, "ws3": ws3t, "ws2": ws2t,
            "scb": tab,
        }
        if ROUTED_MODE == "l1x2":
            xe_l = np.zeros((DIM, C), F8NP)
            xe_l[:, :n_e] = x_lo[:, idx[e]]
            im["xe_l"] = xe_l
        in_maps.append(im)

    key = (C, ROUTED_MODE)
    if key not in _program_cache:
        _program_cache[key] = build_program(C, ROUTED_MODE)
    nc = _program_cache[key]

    res = bass_utils.run_bass_kernel_spmd(
        nc, in_maps, core_ids=list(range(N_CORES)), trace=trace)

    y = np.empty((T, DIM), np.float32)
    for e in range(E):
        sl = slice(TS * e, TS * (e + 1))
        y[sl] = res.results[e]["ys"].T.astype(np.float32)
    for e in range(E):
        ye = res.results[e]["ye"].astype(np.float32)
        y[idx[e]] += ye[:, :len(idx[e])].T * wgt[e][:, None]
    return y.reshape(B, S, DIM), res


def kernel(**inputs) -> np.ndarray:
    out, _ = run(inputs, trace=False)
    return out


# revision 3
# speedup vs baseline: 1.0279x; 1.0279x over previous
"""MoE (DeepSeek-style) routed+shared expert forward on 8 TRN2 NeuronCores.

Strategy (expert-parallel, host-side dispatch):
  - Host computes the gate (softmax + top-2) in float64 and gathers each
    expert's routed tokens; core e processes expert e's tokens (padded to
    capacity C) plus a 1/8 token-slice through the shared-expert MLP.
  - Routed expert matmuls run in fp8(e4m3) with DoubleRow perf mode (two
    128-row k-tiles per instruction, ~1.8x the bf16 rate). The routed
    path contributes only ~23% of the output norm (gate weights ~0.23),
    so fp8 quantization error stays within the accuracy budget.
  - The shared expert (97% of the output norm) runs in float16 (same PE
    rate as bf16, 8x finer mantissa).
  - Dequantization scales fold into activation scale APs; host packs a
    per-core scale/bias table so one SPMD program serves all cores.

ROUTED_MODE:
  "1t"    - all routed matmuls 1-term fp8 (fastest)
  "l1x2"  - layer-1 compensates x with an fp8 residual (2 chains)
  "l2f16" - layer-2 (w2) in f16 instead of fp8
"""

import sys

if "/opt/trn_rl_repo" not in sys.path:
    sys.path.insert(0, "/opt/trn_rl_repo")

import ml_dtypes
import numpy as np

import concourse.bass as bass
import concourse.tile as tile
from concourse import bacc, mybir
from concourse import bass_utils

B, S, DIM = 4, 2048, 1024
T = B * S
INTER = 1024
E = 8
TOPK = 2
ROUTE_SCALE = 1.0
SHARED_INTER = 2048
N_CORES = 8
TS = T // N_CORES          # shared-expert tokens per core
NBLK = 512                 # token block (one fp32 PSUM bank)

F32 = mybir.dt.float32
BF16 = mybir.dt.bfloat16
F16 = mybir.dt.float16
F8 = mybir.dt.float8e4
DR = mybir.MatmulPerfMode.DoubleRow
SILU = mybir.ActivationFunctionType.Silu
IDENT = mybir.ActivationFunctionType.Identity

F8NP = ml_dtypes.float8_e4m3   # IEEE e4m3, max 240

S_X = 16.0    # x quant scale (x absmax ~5.3 -> 85)
S_H = 16.0    # routed intermediate-h quant scale (h absmax ~6 -> 96)

ROUTED_MODE = "1t"

ND = DIM // 128
NI = INTER // 128
NS = SHARED_INTER // 128

_program_cache = {}


def _blocks(total, blk=NBLK):
    out, o = [], 0
    while o < total:
        n = min(blk, total - o)
        out.append((o, n))
        o += n
    return out


def build_program(C, mode):
    nc = bacc.Bacc("TRN2", target_bir_lowering=False, debug=False,
                   num_devices=N_CORES)

    def din(name, shape, dt):
        return nc.dram_tensor(name, shape, dt, kind="ExternalInput").ap()

    xe_h = din("xe_h", (DIM, C), F8)
    xe_l = din("xe_l", (DIM, C), F8) if mode == "l1x2" else None
    w1h = din("w1h", (DIM, INTER), F8)
    w3h = din("w3h", (DIM, INTER), F8)
    w2h = din("w2h", (INTER, DIM), F16 if mode == "l2f16" else F8)
    xs = din("xs", (DIM, TS), F16)
    ws1 = din("ws1", (DIM, SHARED_INTER), F16)
    ws3 = din("ws3", (DIM, SHARED_INTER), F16)
    ws2 = din("ws2", (SHARED_INTER, DIM), F16)
    scb = din("scb", (128, 80), F32)
    ye = nc.dram_tensor("ye", (DIM, C), BF16, kind="ExternalOutput").ap()
    ys = nc.dram_tensor("ys", (DIM, TS), F16, kind="ExternalOutput").ap()
    gate_scr = nc.dram_tensor("gate_scr", (128, 8), F8, kind="Internal").ap()

    def r_in(ap):   # (dk p) n -> p dk n
        return None if ap is None else \
            ap.rearrange("(dk p) n -> p dk n", p=128)

    xe_hr, xe_lr, xs_r = r_in(xe_h), r_in(xe_l), r_in(xs)
    w1r, w3r, w2r = r_in(w1h), r_in(w3h), r_in(w2h)
    ws1r, ws3r, ws2r = r_in(ws1), r_in(ws3), r_in(ws2)
    ye_r = ye.rearrange("(md p) c -> p md c", p=128)
    ys_r = ys.rearrange("(md p) c -> p md c", p=128)

    with tile.TileContext(nc) as tc:
        from contextlib import ExitStack
        es = ExitStack()
        pers = es.enter_context(tc.tile_pool(name="pers", bufs=1,
                                             side="right"))
        scb_sb = pers.tile([128, 80], F32, tag="scb")
        nc.sync.dma_start(scb_sb[:], scb[:])
        inv1, inv3, invy = (scb_sb[:, 0:1], scb_sb[:, 1:2], scb_sb[:, 2:3])
        b1_t = scb_sb[:, 8:8 + NI]
        b3_t = scb_sb[:, 16:16 + NI]       # pre-multiplied by S_H
        b2_t = scb_sb[:, 24:24 + ND]
        bs1_t = scb_sb[:, 32:32 + NS]
        bs3_t = scb_sb[:, 48:48 + NS]
        bs2_t = scb_sb[:, 64:64 + ND]

        def dma_ktiles(dst, src, nk, eng=None):
            eng = eng or nc.sync
            for dk in range(nk):
                eng.dma_start(dst[:, dk, :], src[:, dk, :])

        # shared-expert stationary data lives in a persistent pool and is
        # prefetched while the routed phase computes
        xs_sb = pers.tile([128, ND, TS], F16, tag="xs")
        ws1_sb = pers.tile([128, ND, SHARED_INTER], F16, tag="ws1")
        ws3_sb = pers.tile([128, ND, SHARED_INTER], F16, tag="ws3")
        ws2_sb = pers.tile([128, NS, DIM], F16, tag="ws2")

        # ================= Phase 1: routed expert (fp8 DoubleRow) ======
        with tc.tile_pool(name="wexp", bufs=1) as wpool, \
             tc.tile_pool(name="xep", bufs=1) as xpool, \
             tc.tile_pool(name="hbp", bufs=2) as hpool, \
             tc.tile_pool(name="tmp", bufs=2) as tpool, \
             tc.tile_pool(name="yout", bufs=3) as ypool, \
             tc.tile_pool(name="ps", bufs=2, space="PSUM") as pspool:
            blocks = _blocks(C)
            xeh_sb = xpool.tile([128, ND, C], F8, tag="xe_h")
            w1_sb = wpool.tile([128, ND, INTER], F8, tag="w1h")
            w3_sb = wpool.tile([128, ND, INTER], F8, tag="w3h")
            # weights on the sync ring, x tokens on the gpsimd ring: the
            # two rings fetch concurrently. Full-C x chunks keep DMA
            # segments >= 2KB (per-block chunks would be 512B strided
            # segments that crawl at ~60GB/s).
            # PE warm-up: ~48 tiny matmuls on a zeroed tile raise the PE
            # pstate while the input DMAs stream in (first real matmuls
            # otherwise run ~1.7x slow during the frequency ramp)
            wu = wpool.tile([128, 2, 16], F8, tag="wu")
            nc.vector.memset(wu[:], 0)
            for _ in range(48):
                psw = pspool.tile([16, 16], F32, tag="psw")
                nc.tensor.matmul(psw[:], wu[:], wu[:],
                                 start=True, stop=True, perf_mode=DR)
            for dk in range(ND):
                nc.sync.dma_start(w1_sb[:, dk, :], w1r[:, dk, :])
                nc.sync.dma_start(w3_sb[:, dk, :], w3r[:, dk, :])
                nc.gpsimd.dma_start(xeh_sb[:, dk, :], xe_hr[:, dk, :])
            if mode == "l1x2":
                xel_sb = xpool.tile([128, ND, C], F8, tag="xe_l")
                dma_ktiles(xel_sb, xe_lr, ND, eng=nc.gpsimd)
            w2dt = F16 if mode == "l2f16" else F8
            w2_sb = wpool.tile([128, NI, DIM], w2dt, tag="w2h")
            dma_ktiles(w2_sb, w2r, NI)
            # prefetch shared-phase data on the gpsimd ring, gated behind
            # the last routed inputs (both rings) so it doesn't steal HBM
            # bandwidth from the routed phase's startup
            nc.gpsimd.dma_start(gate_scr[:, 0:4], xeh_sb[:, ND - 1, C - 4:C])
            if mode != "l2f16":
                nc.gpsimd.dma_start(gate_scr[:, 4:8], w2_sb[:, NI - 1, 0:4])
            dma_ktiles(xs_sb, xs_r, ND, eng=nc.gpsimd)
            dma_ktiles(ws1_sb, ws1r, ND, eng=nc.gpsimd)
            dma_ktiles(ws3_sb, ws3r, ND, eng=nc.gpsimd)
            dma_ktiles(ws2_sb, ws2r, NS, eng=nc.gpsimd)

            hdt = F16 if mode == "l2f16" else F8
            for (off, n) in _blocks(C):
                h_sb = hpool.tile([128, NI, n], hdt, tag="h", name="h",
                                  padded_shape=[128, NI, NBLK])
                for mi in range(NI):
                    msl = slice(mi * 128, (mi + 1) * 128)
                    ps1 = pspool.tile([128, n], F32, tag="ps1",
                                      padded_shape=[128, NBLK])
                    ps3 = pspool.tile([128, n], F32, tag="ps3",
                                      padded_shape=[128, NBLK])
                    for ps, w_sb_ in ((ps1, w1_sb), (ps3, w3_sb)):
                        xs_list = [xeh_sb] if mode != "l1x2" \
                            else [xeh_sb, xel_sb]
                        nmm = len(xs_list) * (ND // 2)
                        i = 0
                        for xt in xs_list:
                            for dk in range(0, ND, 2):
                                nc.tensor.matmul(
                                    ps[:], w_sb_[:, dk:dk + 2, msl],
                                    xt[:, dk:dk + 2, off:off + n],
                                    start=(i == 0), stop=(i == nmm - 1),
                                    perf_mode=DR)
                                i += 1
                    t1 = tpool.tile([128, n], BF16, tag="t1", name="t1",
                                    padded_shape=[128, NBLK])
                    nc.scalar.activation(t1[:], ps1[:], SILU,
                                         bias=b1_t[:, mi:mi + 1], scale=inv1)
                    t3 = tpool.tile([128, n], BF16, tag="t3", name="t3",
                                    padded_shape=[128, NBLK])
                    nc.scalar.activation(t3[:], ps3[:], IDENT,
                                         bias=b3_t[:, mi:mi + 1], scale=inv3)
                    nc.vector.tensor_mul(h_sb[:, mi, :], t1[:], t3[:])
                for md in range(ND):
                    msl = slice(md * 128, (md + 1) * 128)
                    psy = pspool.tile([128, n], F32, tag="psy",
                                      padded_shape=[128, NBLK])
                    if mode == "l2f16":
                        for mi in range(NI):
                            nc.tensor.matmul(
                                psy[:], w2_sb[:, mi, msl], h_sb[:, mi, :],
                                start=(mi == 0), stop=(mi == NI - 1))
                    else:
                        for mi in range(0, NI, 2):
                            nc.tensor.matmul(
                                psy[:], w2_sb[:, mi:mi + 2, msl],
                                h_sb[:, mi:mi + 2, :],
                                start=(mi == 0), stop=(mi == NI - 2),
                                perf_mode=DR)
                    yt = ypool.tile([128, n], BF16, tag="yt", name="yt",
                                    padded_shape=[128, NBLK])
                    nc.scalar.activation(yt[:], psy[:], IDENT,
                                         bias=b2_t[:, md:md + 1], scale=invy)
                    nc.sync.dma_start(ye_r[:, md, off:off + n], yt[:])

            # ========== Phase 2: shared expert (f16), same pools ========
            for (off, n) in _blocks(TS):
                hs_sb = hpool.tile([128, NS, n], F16, tag="hs", name="hs",
                                   padded_shape=[128, NS, NBLK])
                for mi in range(NS):
                    msl = slice(mi * 128, (mi + 1) * 128)
                    ps1 = pspool.tile([128, n], F32, tag="ps1",
                                      padded_shape=[128, NBLK])
                    ps3 = pspool.tile([128, n], F32, tag="ps3",
                                      padded_shape=[128, NBLK])
                    for ps, w_sb_ in ((ps1, ws1_sb), (ps3, ws3_sb)):
                        for dk in range(ND):
                            nc.tensor.matmul(
                                ps[:], w_sb_[:, dk, msl],
                                xs_sb[:, dk, off:off + n],
                                start=(dk == 0), stop=(dk == ND - 1))
                    t1 = tpool.tile([128, n], F16, tag="t1s", name="t1s",
                                    padded_shape=[128, NBLK])
                    nc.scalar.activation(t1[:], ps1[:], SILU,
                                         bias=bs1_t[:, mi:mi + 1])
                    t3 = tpool.tile([128, n], F16, tag="t3s", name="t3s",
                                    padded_shape=[128, NBLK])
                    nc.scalar.activation(t3[:], ps3[:], IDENT,
                                         bias=bs3_t[:, mi:mi + 1])
                    nc.vector.tensor_mul(hs_sb[:, mi, :], t1[:], t3[:])
                for md in range(ND):
                    msl = slice(md * 128, (md + 1) * 128)
                    psy = pspool.tile([128, n], F32, tag="psy",
                                      padded_shape=[128, NBLK])
                    for mi in range(NS):
                        nc.tensor.matmul(
                            psy[:], ws2_sb[:, mi, msl], hs_sb[:, mi, :],
                            start=(mi == 0), stop=(mi == NS - 1))
                    yt = ypool.tile([128, n], F16, tag="yts", name="yts",
                                    padded_shape=[128, NBLK])
                    nc.scalar.activation(yt[:], psy[:], IDENT,
                                         bias=bs2_t[:, md:md + 1])
                    nc.sync.dma_start(ys_r[:, md, off:off + n], yt[:])
        es.close()

    nc.compile()
    return nc


def _q8(a):
    return np.clip(a, -240.0, 240.0).astype(F8NP)


def _pow2_scale(a, target=192.0):
    m = float(np.abs(a).max())
    return float(2.0 ** np.floor(np.log2(target / max(m, 1e-30))))


def _pack_cols(vec, tab, c0):
    k = len(vec) // 128
    tab[:, c0:c0 + k] = vec.reshape(k, 128).T


def _gate_host(xt, gate_w, gate_b):
    logits = xt.astype(np.float64) @ gate_w.astype(np.float64).T \
        + gate_b.astype(np.float64)
    m = logits.max(axis=-1, keepdims=True)
    p = np.exp(logits - m)
    scores = p / p.sum(axis=-1, keepdims=True)
    order = np.argsort(-scores, axis=1, kind="stable")
    top_i = order[:, :TOPK]
    top_w = (np.take_along_axis(scores, top_i, axis=1)
             * ROUTE_SCALE).astype(np.float32)
    return top_i, top_w


def run(inputs, trace=False):
    x = np.ascontiguousarray(np.asarray(inputs["x"], dtype=np.float32))
    f32 = lambda k: np.asarray(inputs[k], dtype=np.float32)
    gate_w, gate_b = f32("gate_w"), f32("gate_b")
    w1, b1, w3, b3 = f32("w1"), f32("b1"), f32("w3"), f32("b3")
    w2, b2 = f32("w2"), f32("b2")
    ws1, bs1, ws3, bs3 = f32("ws1"), f32("bs1"), f32("ws3"), f32("bs3")
    ws2, bs2 = f32("ws2"), f32("bs2")

    xt = x.reshape(T, DIM)
    top_i, top_w = _gate_host(xt, gate_w, gate_b)

    idx, wgt = [], []
    for e in range(E):
        toks = np.nonzero((top_i == e).any(axis=1))[0]
        idx.append(toks)
        slot = (top_i[toks] == e)
        wgt.append(top_w[toks][slot])

    cmax = max(len(i) for i in idx)
    C = max(256, (cmax + 31) & ~31)

    xT = np.ascontiguousarray(xt.T)
    x_hi = _q8(xT * S_X)
    if ROUTED_MODE == "l1x2":
        x_lo = _q8(xT * S_X - x_hi.astype(np.float32))
    xs_f16 = xT.astype(np.float16)

    ws1t = np.ascontiguousarray(ws1.T).astype(np.float16)
    ws3t = np.ascontiguousarray(ws3.T).astype(np.float16)
    ws2t = np.ascontiguousarray(ws2.T).astype(np.float16)

    in_maps = []
    for e in range(E):
        n_e = len(idx[e])
        xe_h = np.zeros((DIM, C), F8NP)
        xe_h[:, :n_e] = x_hi[:, idx[e]]
        sl = slice(TS * e, TS * (e + 1))
        s_w1, s_w3 = _pow2_scale(w1[e]), _pow2_scale(w3[e])
        w1h_ = _q8(np.ascontiguousarray(w1[e].T) * s_w1)
        w3h_ = _q8(np.ascontiguousarray(w3[e].T) * s_w3)
        if ROUTED_MODE == "l2f16":
            s_w2 = 1.0
            w2h_ = np.ascontiguousarray(w2[e].T).astype(np.float16)
        else:
            s_w2 = _pow2_scale(w2[e])
            w2h_ = _q8(np.ascontiguousarray(w2[e].T) * s_w2)

        tab = np.zeros((128, 80), np.float32)
        tab[:, 0] = 1.0 / (S_X * s_w1)
        tab[:, 1] = S_H / (S_X * s_w3)
        tab[:, 2] = 1.0 / (S_H * s_w2)
        if ROUTED_MODE == "l2f16":
            tab[:, 1] = 1.0 / (S_X * s_w3)
            tab[:, 2] = 1.0
        _pack_cols(b1[e], tab, 8)
        _pack_cols(b3[e] * (1.0 if ROUTED_MODE == "l2f16" else S_H), tab, 16)
        _pack_cols(b2[e], tab, 24)
        _pack_cols(bs1, tab, 32)
        _pack_cols(bs3, tab, 48)
        _pack_cols(bs2, tab, 64)

        im = {
            "xe_h": xe_h,
            "xs": np.ascontiguousarray(xs_f16[:, sl]),
            "w1h": w1h_, "w3h": w3h_, "w2h": w2h_,
            "ws1": ws1t, "ws3": ws3t, "ws2": ws2t,
            "scb": tab,
        }
        if ROUTED_MODE == "l1x2":
            xe_l = np.zeros((DIM, C), F8NP)
            xe_l[:, :n_e] = x_lo[:, idx[e]]
            im["xe_l"] = xe_l
        in_maps.append(im)

    key = (C, ROUTED_MODE)
    if key not in _program_cache:
        _program_cache[key] = build_program(C, ROUTED_MODE)
    nc = _program_cache[key]

    res = bass_utils.run_bass_kernel_spmd(
        nc, in_maps, core_ids=list(range(N_CORES)), trace=trace)

    y = np.empty((T, DIM), np.float32)
    for e in range(E):
        sl = slice(TS * e, TS * (e + 1))
        y[sl] = res.results[e]["ys"].T.astype(np.float32)
    for e in range(E):
        ye = res.results[e]["ye"].astype(np.float32)
        y[idx[e]] += ye[:, :len(idx[e])].T * wgt[e][:, None]
    return y.reshape(B, S, DIM), res


def kernel(**inputs) -> np.ndarray:
    out, _ = run(inputs, trace=False)
    return out
